# revision 2
# baseline (speedup 1.0000x reference)
"""Trainium2 Bass kernel for nn_BilinearPolicy (dense_mlp).

Math (reference):
  ob = trunk_obs(obs)      : [B,256] -> 2048 -> 2048 -> 2048 -> 16384 (ReLU between)
  dl = trunk_dlt(deltas)   : same shapes, different weights
  pred[b,a] = sum_f ob[b, a*512+f] * dl[b, f*32+a]            : [B, 32]

Strategy:
  * Data-parallel over batch: 8 cores x 512 rows, zero collectives.
  * Feature-major activations on chip ([feat(part), batch(free)]), so the
    torch-layout weights [din, dout] are used directly as matmul lhsT tiles
    and no transposes are ever needed. Inputs are transposed on host.
  * bf16 matmuls with fp32 PSUM accumulation. Biases applied during the
    mandatory PSUM->SBUF eviction on the Scalar engine (Relu / Identity).
  * dl's last-layer weight columns are permuted on host from (f,a) to (a,f)
    ordering, so the bilinear diagonal becomes: elementwise multiply of the
    two [16384, 512] outputs, then a segmented 512-row partition reduction.
    The 4 z-tiles of an action are accumulated on the DVE; one one-hot mask
    matmul per action accumulates pred^T [32, 512] in a single PSUM tile.
  * The PE is the bottleneck (~96% of the bf16 roofline); the remaining
    slack is schedule edges, attacked here:
      - dummy 128-row warm-up matmuls on a memset scratch tile burn the
        DVFS pstate ramp while the first input/weight DMAs are in flight;
      - the first L0 weight chunk is small (2 m-tiles) and the inputs are
        DMA'd per k-tile on both HWDGE queues so the first real matmul
        starts as early as possible;
      - each action's mask matmul is deferred into the middle of the NEXT
        action's matmul stream so the PE never waits on the DVE chain;
      - the last action's evictions run on Scalar and DVE concurrently to
        shorten the end-of-kernel dependency tail.
"""

from contextlib import ExitStack

import numpy as np
import ml_dtypes

B, OBS, H, F, A = 4096, 256, 2048, 512, 32
DOUT = F * A            # 16384
NCORES = 8
BPC = B // NCORES       # 512 batch rows per core
P = 128

KT = [2, 16, 16, 16]    # k-tiles per layer
MT = [16, 16, 16, 128]  # m-tiles per layer
GR = [2, 4, 4, 4]       # m-tiles grouped per weight DMA
NWARM = 10              # pstate warm-up matmuls (128 moving rows each)

BF16 = ml_dtypes.bfloat16

# Filled with the BassKernelResults of the most recent run (for test harness).
LAST_RESULTS = None


def _tile_weight(w, G):
    """[D1, D2] fp32 -> [D2/(128G), 128(k), G*D1] bf16. Slice
    [:, (g*Kt + j)*128 : +128] of group tile mtg is the lhsT for
    k-tile j of m-tile mtg*G+g; every partition line is contiguous."""
    d1, d2 = w.shape
    kt, mt = d1 // P, d2 // P
    wt = w.reshape(kt, P, mt, P).transpose(2, 1, 0, 3)      # [mt, k, j, m]
    wt = wt.reshape(mt // G, G, P, kt * P).transpose(0, 2, 1, 3)
    wt = wt.reshape(mt // G, P, G * kt * P)
    return np.ascontiguousarray(wt.astype(BF16))


def _tile_bias(b):
    """[D2] fp32 -> [128, D2/128] fp32; column mt holds bias for m-tile mt
    as a per-partition scalar."""
    return np.ascontiguousarray(b.reshape(-1, P).T.astype(np.float32))


def _build_program():
    import concourse.bass as bass
    import concourse.tile as tile
    from concourse import bacc, mybir
    from concourse.bass import ts

    dt = mybir.dt
    AF = mybir.ActivationFunctionType

    nc = bacc.Bacc(
        "TRN2",
        target_bir_lowering=False,
        debug=False,
        enable_asserts=True,
        num_devices=NCORES,
    )

    def din(name, shape, dtype):
        return nc.dram_tensor(name, shape, dtype, kind="ExternalInput").ap()

    x_d = {
        "o": din("xo", [P, 2 * BPC], dt.bfloat16),
        "d": din("xd", [P, 2 * BPC], dt.bfloat16),
    }
    w_d = {}
    b_d = {}
    for t in ("o", "d"):
        for l in range(4):
            w_d[t, l] = din(f"{t}w{l}", [MT[l] // GR[l], P, GR[l] * KT[l] * P],
                            dt.bfloat16)
            b_d[t, l] = din(f"{t}b{l}", [P, MT[l]], dt.float32)
    masks_d = din("masks", [P, A * A], dt.bfloat16)
    pred_d = nc.dram_tensor("pred", [A, BPC], dt.float32, kind="ExternalOutput").ap()

    with tile.TileContext(nc) as tc, ExitStack() as ctx:
        const = ctx.enter_context(tc.tile_pool(name="const", bufs=1))
        wp = ctx.enter_context(tc.tile_pool(name="wp", bufs=4))
        act = ctx.enter_context(tc.tile_pool(name="act", bufs=1))
        ev = ctx.enter_context(tc.tile_pool(name="ev", bufs=10))
        ps = ctx.enter_context(tc.tile_pool(name="ps", bufs=7, space="PSUM"))
        psp = ctx.enter_context(tc.tile_pool(name="psp", bufs=1, space="PSUM"))

        # ---- PE pstate warm-up: small matmuls on a memset scratch tile run
        # while the first input/weight DMAs are in flight, so the DVFS clock
        # is fully ramped by the time real data arrives.
        warm = const.tile([P, 2 * P], dt.bfloat16, tag="warm")
        nc.vector.memset(warm[:], 0.0)
        wps = ps.tile([P, BPC], dt.float32, tag="mm", name="warmps")
        for _ in range(NWARM):
            nc.tensor.matmul(wps[:, :P], warm[:, :P], warm[:, P:],
                             start=True, stop=True)

        # ---- Startup DMAs, finest-grained first so the PE starts early.
        # Inputs are split per k-tile and L0 weights stream in 2-m-tile
        # chunks, interleaved across both HWDGE queues in first-use order.
        x_sb = {}
        w0_sb = {}
        chunk = GR[0] * KT[0] * P
        nch = MT[0] // GR[0]
        for t in ("o", "d"):
            x_sb[t] = const.tile([P, 2, BPC], dt.bfloat16,
                                 tag=f"x{t}", name=f"x{t}")
            w0_sb[t] = const.tile([P, nch * chunk], dt.bfloat16,
                                  tag=f"w0{t}", name=f"w0{t}")
        for t in ("o", "d"):
            xv = x_d[t].rearrange("p (k n) -> p k n", n=BPC)
            nc.sync.dma_start(x_sb[t][:, 0, :], xv[:, 0, :])
            nc.scalar.dma_start(w0_sb[t][:, 0:chunk], w_d[t, 0][0])
            nc.scalar.dma_start(x_sb[t][:, 1, :], xv[:, 1, :])
            for c in range(1, nch):
                eng = nc.sync if c % 2 else nc.scalar
                eng.dma_start(w0_sb[t][:, c * chunk:(c + 1) * chunk],
                              w_d[t, 0][c])

        # remaining weight DMAs round-robin over both HWDGE queues
        dma_engs = [nc.sync, nc.scalar]
        rr = [0]

        def wdma(dst, src):
            dma_engs[rr[0] % len(dma_engs)].dma_start(dst, src)
            rr[0] += 1

        # small constants go on the gpsimd SWDGE queue so they never queue
        # ahead of weights
        bias_sb = {}
        for t in ("o", "d"):
            for l in range(4):
                bias_sb[t, l] = const.tile([P, MT[l]], dt.float32,
                                           tag=f"b{t}{l}", name=f"b{t}{l}")
                nc.gpsimd.dma_start(bias_sb[t, l][:], b_d[t, l][:])
        masks_sb = const.tile([P, A * A], dt.bfloat16, tag="masks")
        nc.gpsimd.dma_start(masks_sb[:], masks_d[:])

        # ---- Trunks: layers 0..2 with ReLU, feature-major throughout.
        # The two trunks are interleaved layer-by-layer so the PE has twice
        # the work per phase start, covering the weight-stream warm-up.
        cur = dict(x_sb)
        for l in range(3):
            for t in ("o", "d"):
                out_t = act.tile([P, MT[l], BPC], dt.bfloat16,
                                 tag=f"h{t}{l % 2}", name=f"h{t}{l}")
                for mtg in range(MT[l] // GR[l]):
                    if l == 0:
                        wt = w0_sb[t]
                    else:
                        wt = wp.tile([P, GR[l] * KT[l] * P], dt.bfloat16,
                                     tag="wbig")
                        wdma(wt[:], w_d[t, l][mtg])
                    for g in range(GR[l]):
                        mt = mtg * GR[l] + g
                        wcol = mt if l == 0 else g  # L0 tile is fully resident
                        pt = ps.tile([P, BPC], dt.float32, tag="mm")
                        for j in range(KT[l]):
                            nc.tensor.matmul(
                                pt[:], wt[:, ts(wcol * KT[l] + j, P)],
                                cur[t][:, j, :],
                                start=(j == 0), stop=(j == KT[l] - 1),
                            )
                        nc.scalar.activation(
                            out_t[:, mt, :], pt[:], AF.Relu,
                            bias=bias_sb[t, l][:, mt:mt + 1],
                        )
                cur[t] = out_t
        h = cur

        # ---- Layer 3 + bilinear diagonal, fused per 128-feature tile.
        # Each action's mask matmul is deferred into the middle of the next
        # action's matmul stream so the PE never waits on the DVE chain.
        pred_ps = psp.tile([A, BPC], dt.float32, tag="pred")
        pending = None

        def emit_mask(pa, pz):
            nc.tensor.matmul(
                pred_ps[:], masks_sb[:, ts(pa, A)], pz[:],
                start=(pa == 0), stop=(pa == A - 1),
            )

        for a in range(A):  # one weight DMA per trunk covers the whole action
            last = a == A - 1
            z_acc = ev.tile([P, BPC], dt.bfloat16, tag="zacc")
            wt = {}
            for t in ("o", "d"):
                wt[t] = wp.tile([P, GR[3] * KT[3] * P], dt.bfloat16,
                                tag="wbig", name=f"w3{t}")
                wdma(wt[t][:], w_d[t, 3][a])
            for g in range(GR[3]):
                mt = a * 4 + g
                s = {}
                for t in ("o", "d"):
                    pt = ps.tile([P, BPC], dt.float32, tag="mm")
                    for j in range(KT[3]):
                        nc.tensor.matmul(
                            pt[:], wt[t][:, ts(g * KT[3] + j, P)],
                            h[t][:, j, :],
                            start=(j == 0), stop=(j == KT[3] - 1),
                        )
                    s[t] = ev.tile([P, BPC], dt.bfloat16, tag="evict",
                                   name=f"s{t}")
                    if last and t == "d":
                        # end-of-kernel tail: evict on the otherwise-idle
                        # DVE so Scalar and DVE drain concurrently
                        nc.vector.tensor_scalar_add(
                            s[t][:], pt[:], bias_sb[t, 3][:, mt:mt + 1])
                    else:
                        nc.scalar.activation(
                            s[t][:], pt[:], AF.Identity,
                            bias=bias_sb[t, 3][:, mt:mt + 1],
                        )
                if g == 2 and pending is not None:
                    emit_mask(*pending)
                    pending = None
                if g == 0:
                    nc.vector.tensor_mul(z_acc[:], s["o"][:], s["d"][:])
                else:
                    zt = ev.tile([P, BPC], dt.bfloat16, tag="ztmp")
                    nc.vector.tensor_mul(zt[:], s["o"][:], s["d"][:])
                    nc.vector.tensor_add(z_acc[:], z_acc[:], zt[:])
            pending = (a, z_acc)
        emit_mask(*pending)

        pred_sb = ev.tile([A, BPC], dt.float32, tag="predsb", bufs=1)
        nc.vector.tensor_copy(pred_sb[:], pred_ps[:])
        nc.sync.dma_start(pred_d[:], pred_sb[:])

    nc.compile()
    return nc


def _prep_inputs(inputs):
    """Host-side layout/dtype prep shared across cores + per-core slices."""
    shared = {}

    for t, pfx in (("o", "obs"), ("d", "dlt")):
        for l in range(4):
            w = np.asarray(inputs[f"{pfx}_W{l}"], np.float32)
            b = np.asarray(inputs[f"{pfx}_b{l}"], np.float32)
            if t == "d" and l == 3:
                # permute columns (f,a) -> (a,f) to match obs layout
                w = w.reshape(H, F, A).transpose(0, 2, 1).reshape(H, DOUT)
                b = b.reshape(F, A).T.reshape(DOUT)
            shared[f"{t}w{l}"] = _tile_weight(w, GR[l])
            shared[f"{t}b{l}"] = _tile_bias(b)

    masks = np.zeros((P, A, A), np.float32)
    for a in range(A):
        masks[:, a, a] = 1.0
    shared["masks"] = np.ascontiguousarray(masks.reshape(P, A * A).astype(BF16))

    obsT = np.asarray(inputs["obs"], np.float32).T.astype(BF16)    # [256, 4096]
    dltT = np.asarray(inputs["deltas"], np.float32).T.astype(BF16)

    in_maps = []
    for c in range(NCORES):
        sl = slice(c * BPC, (c + 1) * BPC)
        m = dict(shared)
        m["xo"] = np.ascontiguousarray(
            obsT[:, sl].reshape(2, P, BPC).transpose(1, 0, 2).reshape(P, 2 * BPC))
        m["xd"] = np.ascontiguousarray(
            dltT[:, sl].reshape(2, P, BPC).transpose(1, 0, 2).reshape(P, 2 * BPC))
        in_maps.append(m)
    return in_maps


_PROGRAM = None


def kernel(**inputs):
    global _PROGRAM, LAST_RESULTS
    from concourse.bass_utils import run_bass_kernel_spmd

    if _PROGRAM is None:
        _PROGRAM = _build_program()
    in_maps = _prep_inputs(inputs)
    res = run_bass_kernel_spmd(_PROGRAM, in_maps, list(range(NCORES)))
    LAST_RESULTS = res
    out = np.empty((B, A), np.float32)
    for c in range(NCORES):
        out[c * BPC:(c + 1) * BPC] = res.results[c]["pred"].T
    return out


# revision 4
# speedup vs baseline: 1.0025x; 1.0025x over previous
"""Trainium2 Bass kernel for nn_BilinearPolicy (dense_mlp).

Math (reference):
  ob = trunk_obs(obs)      : [B,256] -> 2048 -> 2048 -> 2048 -> 16384 (ReLU between)
  dl = trunk_dlt(deltas)   : same shapes, different weights
  pred[b,a] = sum_f ob[b, a*512+f] * dl[b, f*32+a]            : [B, 32]

Strategy:
  * Data-parallel over batch: 8 cores x 512 rows, zero collectives.
  * Feature-major activations on chip ([feat(part), batch(free)]), so the
    torch-layout weights [din, dout] are used directly as matmul lhsT tiles
    and no transposes are ever needed. Inputs are transposed on host.
  * bf16 matmuls with fp32 PSUM accumulation. Biases applied during the
    mandatory PSUM->SBUF eviction: trunk o on the Scalar engine, trunk d on
    the DVE (tensor_scalar), so neither queue's serial backlog stalls the PE
    at group boundaries.
  * dl's last-layer weight columns are permuted on host from (f,a) to (a,f)
    ordering, so the bilinear diagonal becomes: elementwise multiply of the
    two [16384, 512] outputs, then a segmented 512-row partition reduction.
    The 4 z-tiles of an action are accumulated on the DVE; one one-hot mask
    matmul per action accumulates pred^T [32, 512] in a single PSUM tile.
  * The PE is the bottleneck (~96% of the bf16 roofline); the remaining
    slack is schedule edges, attacked here:
      - the first L0 weight chunk is small (2 m-tiles) and the inputs are
        DMA'd per k-tile on both HWDGE queues so the first real matmul
        starts as early as possible;
      - each action's mask matmul is deferred into the middle of the NEXT
        action's matmul stream so the PE never waits on the DVE chain;
      - the last action keeps per-g z tiles and issues one mask matmul per
        g (deferred by one g), shortening the end-of-kernel tail;
      - pred is copied/DMA'd in two halves: actions 0-15 right after
        mask(15) lands (free, mid-kernel), 16-31 at the end.
"""

from contextlib import ExitStack

import numpy as np
import ml_dtypes

B, OBS, H, F, A = 4096, 256, 2048, 512, 32
DOUT = F * A            # 16384
NCORES = 8
BPC = B // NCORES       # 512 batch rows per core
P = 128

KT = [2, 16, 16, 16]    # k-tiles per layer
MT = [16, 16, 16, 128]  # m-tiles per layer
GR = [2, 4, 4, 4]       # m-tiles grouped per weight DMA

BF16 = ml_dtypes.bfloat16

# Filled with the BassKernelResults of the most recent run (for test harness).
LAST_RESULTS = None


def _tile_weight(w, G):
    """[D1, D2] fp32 -> [D2/(128G), 128(k), G*D1] bf16. Slice
    [:, (g*Kt + j)*128 : +128] of group tile mtg is the lhsT for
    k-tile j of m-tile mtg*G+g; every partition line is contiguous."""
    d1, d2 = w.shape
    kt, mt = d1 // P, d2 // P
    wt = w.reshape(kt, P, mt, P).transpose(2, 1, 0, 3)      # [mt, k, j, m]
    wt = wt.reshape(mt // G, G, P, kt * P).transpose(0, 2, 1, 3)
    wt = wt.reshape(mt // G, P, G * kt * P)
    return np.ascontiguousarray(wt.astype(BF16))


def _tile_bias(b):
    """[D2] fp32 -> [128, D2/128] fp32; column mt holds bias for m-tile mt
    as a per-partition scalar."""
    return np.ascontiguousarray(b.reshape(-1, P).T.astype(np.float32))


def _build_program():
    import concourse.bass as bass
    import concourse.tile as tile
    from concourse import bacc, mybir
    from concourse.alu_op_type import AluOpType
    from concourse.bass import ts

    dt = mybir.dt
    AF = mybir.ActivationFunctionType

    nc = bacc.Bacc(
        "TRN2",
        target_bir_lowering=False,
        debug=False,
        enable_asserts=True,
        num_devices=NCORES,
    )

    def din(name, shape, dtype):
        return nc.dram_tensor(name, shape, dtype, kind="ExternalInput").ap()

    x_d = {
        "o": din("xo", [P, 2 * BPC], dt.bfloat16),
        "d": din("xd", [P, 2 * BPC], dt.bfloat16),
    }
    w_d = {}
    b_d = {}
    for t in ("o", "d"):
        for l in range(4):
            w_d[t, l] = din(f"{t}w{l}", [MT[l] // GR[l], P, GR[l] * KT[l] * P],
                            dt.bfloat16)
            b_d[t, l] = din(f"{t}b{l}", [P, MT[l]], dt.float32)
    masks_d = din("masks", [P, A * A], dt.bfloat16)
    pred_d = nc.dram_tensor("pred", [A, BPC], dt.float32, kind="ExternalOutput").ap()

    with tile.TileContext(nc) as tc, ExitStack() as ctx:
        const = ctx.enter_context(tc.tile_pool(name="const", bufs=1))
        wp = ctx.enter_context(tc.tile_pool(name="wp", bufs=4))
        act = ctx.enter_context(tc.tile_pool(name="act", bufs=1))
        ev = ctx.enter_context(tc.tile_pool(name="ev", bufs=10))
        ps = ctx.enter_context(tc.tile_pool(name="ps", bufs=7, space="PSUM"))
        psp = ctx.enter_context(tc.tile_pool(name="psp", bufs=1, space="PSUM"))

        # ---- Startup DMAs, finest-grained first so the PE starts early.
        # Inputs are split per k-tile and L0 weights stream in 2-m-tile
        # chunks, interleaved across both HWDGE queues in first-use order.
        x_sb = {}
        w0_sb = {}
        chunk = GR[0] * KT[0] * P
        nch = MT[0] // GR[0]
        for t in ("o", "d"):
            x_sb[t] = const.tile([P, 2, BPC], dt.bfloat16,
                                 tag=f"x{t}", name=f"x{t}")
            w0_sb[t] = const.tile([P, nch * chunk], dt.bfloat16,
                                  tag=f"w0{t}", name=f"w0{t}")
        for t in ("o", "d"):
            xv = x_d[t].rearrange("p (k n) -> p k n", n=BPC)
            nc.sync.dma_start(x_sb[t][:, 0, :], xv[:, 0, :])
            nc.scalar.dma_start(w0_sb[t][:, 0:chunk], w_d[t, 0][0])
            nc.scalar.dma_start(x_sb[t][:, 1, :], xv[:, 1, :])
            for c in range(1, nch):
                eng = nc.sync if c % 2 else nc.scalar
                eng.dma_start(w0_sb[t][:, c * chunk:(c + 1) * chunk],
                              w_d[t, 0][c])

        # remaining weight DMAs round-robin over both HWDGE queues
        dma_engs = [nc.sync, nc.scalar]
        rr = [0]

        def wdma(dst, src):
            dma_engs[rr[0] % len(dma_engs)].dma_start(dst, src)
            rr[0] += 1

        # small constants go on the gpsimd SWDGE queue so they never queue
        # ahead of weights
        bias_sb = {}
        for t in ("o", "d"):
            for l in range(4):
                bias_sb[t, l] = const.tile([P, MT[l]], dt.float32,
                                           tag=f"b{t}{l}", name=f"b{t}{l}")
                nc.gpsimd.dma_start(bias_sb[t, l][:], b_d[t, l][:])
        masks_sb = const.tile([P, A * A], dt.bfloat16, tag="masks")
        nc.gpsimd.dma_start(masks_sb[:], masks_d[:])

        def evict(t, dst, pt, l, mt, relu):
            """PSUM->SBUF with bias: trunk o on Scalar, trunk d on DVE."""
            bias = bias_sb[t, l][:, mt:mt + 1]
            if t == "o":
                nc.scalar.activation(dst, pt, AF.Relu if relu else AF.Identity,
                                     bias=bias)
            elif relu:
                nc.vector.tensor_scalar(dst, pt, bias, 0.0,
                                        AluOpType.add, AluOpType.max)
            else:
                nc.vector.tensor_scalar_add(dst, pt, bias)

        # ---- Trunks: layers 0..2 with ReLU, feature-major throughout.
        # The two trunks are interleaved layer-by-layer so the PE has twice
        # the work per phase start, covering the weight-stream warm-up.
        cur = dict(x_sb)
        for l in range(3):
            for t in ("o", "d"):
                out_t = act.tile([P, MT[l], BPC], dt.bfloat16,
                                 tag=f"h{t}{l % 2}", name=f"h{t}{l}")
                for mtg in range(MT[l] // GR[l]):
                    if l == 0:
                        wt = w0_sb[t]
                    else:
                        wt = wp.tile([P, GR[l] * KT[l] * P], dt.bfloat16,
                                     tag="wbig")
                        wdma(wt[:], w_d[t, l][mtg])
                    for g in range(GR[l]):
                        mt = mtg * GR[l] + g
                        wcol = mt if l == 0 else g  # L0 tile is fully resident
                        pt = ps.tile([P, BPC], dt.float32, tag="mm")
                        for j in range(KT[l]):
                            nc.tensor.matmul(
                                pt[:], wt[:, ts(wcol * KT[l] + j, P)],
                                cur[t][:, j, :],
                                start=(j == 0), stop=(j == KT[l] - 1),
                            )
                        evict(t, out_t[:, mt, :], pt[:], l, mt, relu=True)
                cur[t] = out_t
        h = cur

        # ---- Layer 3 + bilinear diagonal, fused per 128-feature tile.
        # Mask matmuls are deferred into the middle of the next action's
        # matmul stream so the PE never waits on the DVE chain. pred is
        # copied/DMA'd out in two halves (mid-kernel and end).
        pred_ps = psp.tile([A, BPC], dt.float32, tag="pred")
        pred_sb = ev.tile([A, BPC], dt.float32, tag="predsb", bufs=1)
        pending = []

        def emit_mask(pa, pz, start, stop):
            nc.tensor.matmul(
                pred_ps[:], masks_sb[:, ts(pa, A)], pz[:],
                start=start, stop=stop,
            )

        for a in range(A):  # one weight DMA per trunk covers the whole action
            last = a == A - 1
            z_acc = None
            wt = {}
            for t in ("o", "d"):
                wt[t] = wp.tile([P, GR[3] * KT[3] * P], dt.bfloat16,
                                tag="wbig", name=f"w3{t}")
                wdma(wt[t][:], w_d[t, 3][a])
            for g in range(GR[3]):
                mt = a * 4 + g
                s = {}
                for t in ("o", "d"):
                    pt = ps.tile([P, BPC], dt.float32, tag="mm")
                    for j in range(KT[3]):
                        nc.tensor.matmul(
                            pt[:], wt[t][:, ts(g * KT[3] + j, P)],
                            h[t][:, j, :],
                            start=(j == 0), stop=(j == KT[3] - 1),
                        )
                    s[t] = ev.tile([P, BPC], dt.bfloat16, tag="evict",
                                   name=f"s{t}")
                    evict(t, s[t][:], pt[:], 3, mt, relu=False)
                if g == 1 and pending:
                    emit_mask(*pending.pop(0))
                    if a == 16:
                        # actions 0-15 are final in pred partitions 0:16 —
                        # copy + DMA them out now, off the critical tail
                        nc.scalar.activation(pred_sb[0:16, :], pred_ps[0:16, :],
                                             AF.Copy)
                        nc.sync.dma_start(pred_d[0:16, :], pred_sb[0:16, :])
                if last:
                    # per-g mask matmuls (deferred by one g) keep the final
                    # dependency chain one DVE mult + one matmul long
                    zg = ev.tile([P, BPC], dt.bfloat16, tag="ztmp",
                                 name="zg")
                    nc.vector.tensor_mul(zg[:], s["o"][:], s["d"][:])
                    pending.append((a, zg, False, False))
                    if g == GR[3] - 1:
                        while len(pending) > 1:
                            emit_mask(*pending.pop(0))
                        pa, pz, st, _ = pending.pop(0)
                        emit_mask(pa, pz, st, True)
                elif g == 0:
                    z_acc = ev.tile([P, BPC], dt.bfloat16, tag="zacc")
                    nc.vector.tensor_mul(z_acc[:], s["o"][:], s["d"][:])
                else:
                    zt = ev.tile([P, BPC], dt.bfloat16, tag="ztmp")
                    nc.vector.tensor_mul(zt[:], s["o"][:], s["d"][:])
                    nc.vector.tensor_add(z_acc[:], z_acc[:], zt[:])
            if not last:
                pending.append((a, z_acc, a == 0, False))

        # engine APs must start at a 32-aligned partition, so the tail copy
        # re-reads the whole bank; only the DMA is split
        nc.scalar.activation(pred_sb[:], pred_ps[:], AF.Copy)
        nc.sync.dma_start(pred_d[16:32, :], pred_sb[16:32, :])

    nc.compile()
    return nc


def _prep_inputs(inputs):
    """Host-side layout/dtype prep shared across cores + per-core slices."""
    shared = {}

    for t, pfx in (("o", "obs"), ("d", "dlt")):
        for l in range(4):
            w = np.asarray(inputs[f"{pfx}_W{l}"], np.float32)
            b = np.asarray(inputs[f"{pfx}_b{l}"], np.float32)
            if t == "d" and l == 3:
                # permute columns (f,a) -> (a,f) to match obs layout
                w = w.reshape(H, F, A).transpose(0, 2, 1).reshape(H, DOUT)
                b = b.reshape(F, A).T.reshape(DOUT)
            shared[f"{t}w{l}"] = _tile_weight(w, GR[l])
            shared[f"{t}b{l}"] = _tile_bias(b)

    masks = np.zeros((P, A, A), np.float32)
    for a in range(A):
        masks[:, a, a] = 1.0
    shared["masks"] = np.ascontiguousarray(masks.reshape(P, A * A).astype(BF16))

    obsT = np.asarray(inputs["obs"], np.float32).T.astype(BF16)    # [256, 4096]
    dltT = np.asarray(inputs["deltas"], np.float32).T.astype(BF16)

    in_maps = []
    for c in range(NCORES):
        sl = slice(c * BPC, (c + 1) * BPC)
        m = dict(shared)
        m["xo"] = np.ascontiguousarray(
            obsT[:, sl].reshape(2, P, BPC).transpose(1, 0, 2).reshape(P, 2 * BPC))
        m["xd"] = np.ascontiguousarray(
            dltT[:, sl].reshape(2, P, BPC).transpose(1, 0, 2).reshape(P, 2 * BPC))
        in_maps.append(m)
    return in_maps


_PROGRAM = None


def kernel(**inputs):
    global _PROGRAM, LAST_RESULTS
    from concourse.bass_utils import run_bass_kernel_spmd

    if _PROGRAM is None:
        _PROGRAM = _build_program()
    in_maps = _prep_inputs(inputs)
    res = run_bass_kernel_spmd(_PROGRAM, in_maps, list(range(NCORES)))
    LAST_RESULTS = res
    out = np.empty((B, A), np.float32)
    for c in range(NCORES):
        out[c * BPC:(c + 1) * BPC] = res.results[c]["pred"].T
    return out
